# revision 11
# baseline (speedup 1.0000x reference)
"""Causal self-attention (B=8, T=1024, C=768, NH=12) on 8 TRN2 NeuronCores.

Strategy: pure batch data-parallel — core b computes batch element b end to
end (no collectives). All big matmuls run in fp32r (full-rate fp32 on the PE).

Per-core dataflow (everything kept "transposed", i.e. [feature, time]):
  xT [C, T]                                  (host pre-transposes x[b])
  qkT[j, t] = Wqk[:, j].T x  (j in 0..1536)  attT-friendly layout
  v  [t, j] = x Wv           (j in 0..768)   AV-friendly layout, augmented
                                             with a ones column per head
  attT[tk, tq] = kT.T @ qT   per head        PSUM [128, 512] tiles
  expT = exp(scale * attT)   (no max-sub: |logits|<~3 for this problem)
  out_aug[d + {sum}, tq] = [v | 1].T @ expT  ones column => softmax denom
  rawT[j, t] = out_aug[d] * (1/denom)        denom broadcast via 0-stride DMA
  yT[e, t] = Wp.T @ rawT + bp                output, host transposes back
"""

import os
import sys

import numpy as np

for _p in ("/opt/trn_rl_repo", "/root/.axon_site/_ro/trn_rl_repo"):
    if os.path.isdir(_p) and _p not in sys.path:
        sys.path.insert(0, _p)

import concourse.bacc as bacc
import concourse.mybir as mybir
import concourse.tile as tile
from concourse.bass import ts
from concourse.bass_utils import run_bass_kernel_spmd

B, T, C = 8, 1024, 768
NH, HD = 12, 64
P = 128
NCORES = 8
CC = C // P            # 6 contraction chunks over C
JQK = 2 * C // P       # 12 output chunks for q|k
EC = C // P            # 6 output chunks for the projection
TQ = 512               # moving-dim tile (max for 4-byte matmul)
NTQ = T // TQ          # 2
NTK = T // P           # 8 key chunks
G = NH // 2            # 6 head pairs (two 64-wide heads per 128 partitions)
VW = 2 * HD + 2        # 130: per-pair v layout [d_even(64), 1, 1, d_odd(64)]
SCALE = 1.0 / float(np.sqrt(HD))
F32 = mybir.dt.float32
F32R = mybir.dt.float32r
AF = mybir.ActivationFunctionType

_CACHE = {}


def _r(ap):
    return ap  # tiles feeding matmuls are already float32r


def _build():
    if "nc" in _CACHE:
        return _CACHE["nc"]

    nc = bacc.Bacc("TRN2", target_bir_lowering=False, debug=False)

    xT = nc.dram_tensor("xT", [C, T], F32R, kind="ExternalInput")
    wqk = nc.dram_tensor("wqk", [C, 2 * C], F32R, kind="ExternalInput")
    wv = nc.dram_tensor("wv", [C, C], F32R, kind="ExternalInput")
    wp = nc.dram_tensor("wp", [C, C], F32R, kind="ExternalInput")
    bqk = nc.dram_tensor("bqk", [P, JQK], F32, kind="ExternalInput")
    bvr = nc.dram_tensor("bvr", [P, C], F32, kind="ExternalInput")
    bp = nc.dram_tensor("bp", [P, EC], F32, kind="ExternalInput")
    tri = nc.dram_tensor("tri", [P, P], F32, kind="ExternalInput")
    yT = nc.dram_tensor("yT", [C, T], F32, kind="ExternalOutput")

    xT_r = xT[:].rearrange("(o p) t -> p o t", p=P)
    wqk_r = wqk[:].rearrange("(o p) j -> p o j", p=P)
    wv_r = wv[:].rearrange("(o p) j -> p o j", p=P)
    wp_r = wp[:].rearrange("(o p) e -> p o e", p=P)
    yT_r = yT[:].rearrange("(o p) t -> p o t", p=P)

    with tile.TileContext(nc) as tc:
        with (
            tc.tile_pool(name="const", bufs=1) as constp,
            tc.tile_pool(name="xt", bufs=1) as xtp,
            tc.tile_pool(name="wqk", bufs=3) as wqkp,
            tc.tile_pool(name="wv", bufs=1) as wvp,
            tc.tile_pool(name="qkt", bufs=1) as qkTp,
            tc.tile_pool(name="vaug", bufs=1) as vap,
            tc.tile_pool(name="raw", bufs=1) as rawp,
            tc.tile_pool(name="wp", bufs=2) as wpp,
            tc.tile_pool(name="exp", bufs=6) as expp,
            tc.tile_pool(name="rr", bufs=3) as rrp,
            tc.tile_pool(name="yt", bufs=3) as ytp,
            tc.tile_pool(name="ps", bufs=8, space="PSUM") as psp,
        ):
            # ---- resident tensors -------------------------------------
            xT_sb = xtp.tile([P, CC, T], F32R)
            nc.sync.dma_start(xT_sb[:], xT_r)

            bqk_sb = constp.tile([P, JQK], F32)
            nc.sync.dma_start(bqk_sb[:], bqk[:])
            bv_sb = constp.tile([P, C], F32)
            nc.sync.dma_start(bv_sb[:], bvr[:])
            bp_sb = constp.tile([P, EC], F32)
            nc.sync.dma_start(bp_sb[:], bp[:])
            tri_sb = constp.tile([P, P], F32)
            nc.sync.dma_start(tri_sb[:], tri[:])

            qkT_sb = qkTp.tile([P, JQK, T], F32R)
            v_sb = vap.tile([P, NTK, G, VW], F32R)
            rawT = rawp.tile([P, CC, T], F32R)

            # ---- qkT[j, t] = Wqk.T x + b ------------------------------
            for jc in range(JQK):
                wt = wqkp.tile([P, CC, P], F32R, tag="wqk")
                nc.sync.dma_start(wt[:], wqk_r[:, :, ts(jc, P)])
                for t2 in range(NTQ):
                    ps = psp.tile([P, TQ], F32, tag="bank")
                    for cc in range(CC):
                        nc.tensor.matmul(
                            ps[:],
                            _r(wt[:, cc, :]),
                            _r(xT_sb[:, cc, ts(t2, TQ)]),
                            start=(cc == 0),
                            stop=(cc == CC - 1),
                        )
                    nc.scalar.activation(
                        qkT_sb[:, jc, ts(t2, TQ)],
                        ps[:],
                        AF.Identity,
                        bias=bqk_sb[:, jc : jc + 1],
                    )

            # ---- v[t, j] = x Wv + b, interleaved per head pair --------
            # ones columns for the softmax-denominator trick
            # pair layout: [d_even(64), 1, d_odd(64), 1]
            # (memset can't write fp32r -> broadcast-copy from a f32 const)
            onec = constp.tile([P, 1], F32)
            nc.vector.memset(onec[:], 1.0)
            ones_src = onec[:, None, None, :].to_broadcast([P, NTK, G, 1])
            nc.any.tensor_copy(v_sb[:, :, :, HD : HD + 1], ones_src)
            nc.any.tensor_copy(v_sb[:, :, :, VW - 1 : VW], ones_src)
            wv_sb = wvp.tile([P, CC, C], F32R)
            nc.sync.dma_start(wv_sb[:], wv_r)
            JV = 384  # v output tile width
            for tc_i in range(NTK):
                for jn in range(C // JV):
                    ps = psp.tile([P, TQ], F32, tag="bank")
                    for cc in range(CC):
                        nc.tensor.matmul(
                            ps[:, :JV],
                            _r(xT_sb[:, cc, ts(tc_i, P)]),
                            _r(wv_sb[:, cc, ts(jn, JV)]),
                            start=(cc == 0),
                            stop=(cc == CC - 1),
                        )
                    g0 = jn * (JV // P)  # 3 head pairs per 384 cols
                    src = ps[:, :JV].rearrange("p (g h d) -> p g h d", h=2, d=HD)
                    bias = bv_sb[:, ts(jn, JV)].rearrange(
                        "p (g h d) -> p g h d", h=2, d=HD
                    )
                    nc.vector.tensor_tensor(
                        v_sb[:, tc_i, g0 : g0 + 3, 0:HD],
                        src[:, :, 0, :],
                        bias[:, :, 0, :],
                        mybir.AluOpType.add,
                    )
                    nc.vector.tensor_tensor(
                        v_sb[:, tc_i, g0 : g0 + 3, HD + 1 : VW - 1],
                        src[:, :, 1, :],
                        bias[:, :, 1, :],
                        mybir.AluOpType.add,
                    )

            # ---- attention per head, head pair shares recip tile ------
            for g in range(G):
                jq_even, jk_even = g, G + g
                for t2 in range(NTQ):
                    hi = 4 * (t2 + 1)  # causal: key chunks 0..hi-1
                    avs = []
                    for parity in (0, 1):
                        qrow = HD * parity
                        av = psp.tile([P, TQ], F32, tag="bank")
                        avs.append(av)
                        av_out = av[0:65, :]  # rows 0:64 = d, row 64 = denom
                        vlo = (HD + 1) * parity
                        for tkc in range(hi):
                            # columns < cs are fully masked: never compute,
                            # exp, or accumulate them (tkc==0 has cs==0, so
                            # the start=True matmul covers the full tile)
                            csr = tkc * P - t2 * TQ  # diag block start col
                            cs = max(0, csr)
                            pa = psp.tile([P, TQ], F32, tag="bank")
                            nc.tensor.matmul(
                                pa[:, cs:],
                                _r(qkT_sb[qrow : qrow + HD, jk_even, ts(tkc, P)]),
                                _r(
                                    qkT_sb[
                                        qrow : qrow + HD,
                                        jq_even,
                                        t2 * TQ + cs : (t2 + 1) * TQ,
                                    ]
                                ),
                                start=True,
                                stop=True,
                            )
                            e = expp.tile([P, TQ], F32R, tag="exp")
                            nc.scalar.activation(
                                e[:, cs:], pa[:, cs:], AF.Exp, scale=SCALE
                            )
                            if csr >= 0:
                                # diagonal 128-block needs the causal mask
                                nc.vector.tensor_mul(
                                    e[:, cs : cs + P],
                                    e[:, cs : cs + P],
                                    tri_sb[:],
                                )
                            nc.tensor.matmul(
                                av_out[:, cs:],
                                _r(v_sb[:, tkc, g, vlo : vlo + HD + 1]),
                                _r(e[:, cs:]),
                                start=(tkc == 0),
                                stop=(tkc == hi - 1),
                            )
                    # denominators -> reciprocal -> partition broadcast.
                    # DVE lanes can't cross partitions, so everything runs at
                    # base 0; the odd head's result reaches partitions 64:128
                    # of rawT via an SBUF->SBUF DMA (address-based, can cross).
                    for parity in (0, 1):
                        rcp = rrp.tile([P, TQ], F32, tag="rcp")
                        nc.vector.reciprocal(
                            rcp[64:65, :], avs[parity][64:65, :]
                        )
                        rr = rrp.tile([64, TQ], F32, tag="rr")
                        nc.sync.dma_start(
                            rr[:],
                            rcp[64:65, None, :].to_broadcast([1, 64, TQ]),
                        )
                        if parity == 0:
                            nc.vector.tensor_mul(
                                rawT[0:64, g, ts(t2, TQ)],
                                avs[0][0:64, :],
                                rr[:],
                            )
                        else:
                            tmp = rrp.tile([64, TQ], F32R, tag="otmp")
                            nc.vector.tensor_mul(
                                tmp[:], avs[1][0:64, :], rr[:]
                            )
                            nc.sync.dma_start(
                                rawT[64:128, g, ts(t2, TQ)], tmp[:]
                            )

            # ---- yT[e, t] = Wp.T rawT + bp ----------------------------
            for ec in range(EC):
                wpt = wpp.tile([P, CC, P], F32R, tag="wp")
                nc.sync.dma_start(wpt[:], wp_r[:, :, ts(ec, P)])
                for t2 in range(NTQ):
                    ps = psp.tile([P, TQ], F32, tag="bank")
                    for jc in range(CC):
                        nc.tensor.matmul(
                            ps[:],
                            _r(wpt[:, jc, :]),
                            _r(rawT[:, jc, ts(t2, TQ)]),
                            start=(jc == 0),
                            stop=(jc == CC - 1),
                        )
                    yt = ytp.tile([P, TQ], F32, tag="yt")
                    nc.scalar.activation(
                        yt[:], ps[:], AF.Identity, bias=bp_sb[:, ec : ec + 1]
                    )
                    nc.sync.dma_start(yT_r[:, ec, ts(t2, TQ)], yt[:])

    nc.compile()
    _CACHE["nc"] = nc
    return nc


def _round_fp32r(a):
    """Round fp32 to fp32r (11-bit mantissa) the way the PE expects."""
    u = np.ascontiguousarray(a, dtype=np.float32).view(np.uint32)
    u = ((u.astype(np.uint64) + 0x800) & 0xFFFFF000).astype(np.uint32)
    return u.view(np.float32)


def make_in_maps(x, w_attn, b_attn, w_proj, b_proj):
    x = np.ascontiguousarray(np.asarray(x, dtype=np.float32))
    w_attn = np.ascontiguousarray(np.asarray(w_attn, dtype=np.float32))
    b_attn = np.ascontiguousarray(np.asarray(b_attn, dtype=np.float32))
    w_proj = np.ascontiguousarray(np.asarray(w_proj, dtype=np.float32))
    b_proj = np.ascontiguousarray(np.asarray(b_proj, dtype=np.float32))

    wqk = _round_fp32r(w_attn[:, : 2 * C])
    wv = _round_fp32r(w_attn[:, 2 * C :])
    w_proj = _round_fp32r(w_proj)
    bqk = np.ascontiguousarray(b_attn[: 2 * C].reshape(JQK, P).T)
    bvr = np.ascontiguousarray(np.tile(b_attn[2 * C :][None, :], (P, 1)))
    bp = np.ascontiguousarray(b_proj.reshape(EC, P).T)
    tri = np.triu(np.ones((P, P), dtype=np.float32))  # keep col >= row

    shared = {
        "wqk": wqk,
        "wv": wv,
        "wp": w_proj,
        "bqk": bqk,
        "bvr": bvr,
        "bp": bp,
        "tri": tri,
    }
    return [
        {"xT": _round_fp32r(x[b].T), **shared} for b in range(NCORES)
    ]


def kernel(**inputs):
    nc = _build()
    in_maps = make_in_maps(
        inputs["x"],
        inputs["w_attn"],
        inputs["b_attn"],
        inputs["w_proj"],
        inputs["b_proj"],
    )
    res = run_bass_kernel_spmd(nc, in_maps, list(range(NCORES)))
    out = np.stack(
        [np.ascontiguousarray(res.results[b]["yT"].T) for b in range(NCORES)]
    )
    return out.astype(np.float32)
